# revision 1
# baseline (speedup 1.0000x reference)
"""Trainium2 Bass kernel for a single attention head (B=4, S=2048, D=4096, DH=128).

Sharding: 8 cores = (batch b, q-half h). Each core:
  - computes K^T, V^T for the full sequence of its batch (duplicated across the
    2 cores of a batch), Q^T for its own 1024-row half,
  - logits (bf16-rounded, matching the reference) + mask -> softmax (fp32
    stats) -> bf16 weights -> out = W @ V, scaled by 1/rowsum.

Key layout trick: K/V sequence columns are PERMUTED on the host so each core's
own q-half comes first; the Q projection then always reads columns [0,1024) of
xT, keeping the SPMD graph identical across cores. Softmax/PV are invariant to
a consistent key permutation.

Phase structure (PSUM is 8 banks of [128,512]f32):
  1a: per d-chunk i: one [128,2048] x tile feeds K(x4) + Q(x2) + V(x2 first
      half) 512-wide matmuls -- 8 PSUM accumulator banks, x read once.
  1b: V second half from re-read [128,1024] x tiles (2 banks).
  2:  per q-tile: logits -> one fused DVE add (PSUM f32 + mask -> bf16, which
      applies the reference's bf16 logits rounding) -> row max -> exp (ACT,
      bias=-max, accumulates row sum) -> W^T via DMA xbar transpose -> PV,
      software-pipelined so PE stays busy across q-tiles.
"""

import numpy as np
import ml_dtypes

import concourse.bass as bass
import concourse.tile as tile
from concourse import bacc, mybir
from concourse.bass_utils import run_bass_kernel_spmd

B, S, D, DH = 4, 2048, 4096, 128
SQ = S // 2          # q rows per core
N_CORES = 8
D_CH = D // 128      # 32 contraction chunks
QT_TILES = SQ // 128 # 8 q row tiles
K_CH = S // 128      # 16 key chunks for PV

BF16 = mybir.dt.bfloat16
F32 = mybir.dt.float32


def build_nc():
    nc = bacc.Bacc(None)

    xT = nc.dram_tensor("xT", [D, S], BF16, kind="ExternalInput")
    mask = nc.dram_tensor("mask", [SQ, S], BF16, kind="ExternalInput")
    # weights pre-tiled on host: w[p, i, m] = W[m, i*128+p]
    wqT = nc.dram_tensor("wqT", [128, D_CH, DH], BF16, kind="ExternalInput")
    wkT = nc.dram_tensor("wkT", [128, D_CH, DH], BF16, kind="ExternalInput")
    wvT = nc.dram_tensor("wvT", [128, D_CH, DH], BF16, kind="ExternalInput")
    bq = nc.dram_tensor("bq", [DH, 1], F32, kind="ExternalInput")
    bk = nc.dram_tensor("bk", [DH, 1], F32, kind="ExternalInput")
    bv = nc.dram_tensor("bv", [DH, 1], F32, kind="ExternalInput")
    out = nc.dram_tensor("out", [SQ, DH], BF16, kind="ExternalOutput")

    with tile.TileContext(nc) as tc:
        with (
            tc.tile_pool(name="weights", bufs=1) as wpool,
            tc.tile_pool(name="persist", bufs=1) as persist,
        ):
            w_sb = {}
            for name, ext in (("q", wqT), ("k", wkT), ("v", wvT)):
                w_sb[name] = wpool.tile([128, D_CH, DH], BF16, tag=f"w{name}",
                                        name=f"w{name}")
            # weights + biases on the scalar HWDGE queue so the sync queue
            # streams x tiles from instruction 0
            for sl in range(4):
                for name, ext in (("k", wkT), ("q", wqT), ("v", wvT)):
                    ss = np.s_[:, sl * 8:(sl + 1) * 8, :]
                    nc.scalar.dma_start(out=w_sb[name][ss], in_=ext[ss])
            b_sb = {}
            for name, ext in (("q", bq), ("k", bk), ("v", bv)):
                t = wpool.tile([DH, 1], F32, tag=f"b{name}")
                nc.scalar.dma_start(out=t[:], in_=ext[:])
                b_sb[name] = t

            kt_sb = persist.tile([DH, S], BF16, tag="kt")    # K^T
            vt_sb = persist.tile([DH, S], BF16, tag="vt")    # V^T (pre-transpose)
            qt_sb = persist.tile([DH, SQ], BF16, tag="qt")   # Q^T
            v_sb = persist.tile([128, K_CH, DH], BF16, tag="v")  # V[kc*128+p, d]

            # --- phase 1a: K (full) + Q (full) + V (first half), 8 PSUM banks ---
            with tc.tile_pool(name="ppsum_a", bufs=1, space="PSUM") as ppa:
                acc = {}
                for tag, n in (("pk", 4), ("pq", 2), ("pv", 2)):
                    for j in range(n):
                        acc[(tag, j)] = ppa.tile([DH, 512], F32, tag=f"{tag}{j}", name=f"{tag}{j}")
                with tc.tile_pool(name="xin_a", bufs=6) as xpa:
                    for i in range(D_CH):
                        xt = xpa.tile([128, S], BF16, tag="xt")
                        nc.sync.dma_start(out=xt[:], in_=xT[i * 128:(i + 1) * 128, :])
                        st = dict(start=(i == 0), stop=(i == D_CH - 1))
                        for j in range(4):
                            nc.tensor.matmul(acc[("pk", j)][:], lhsT=w_sb["k"][:, i, :],
                                             rhs=xt[:, j * 512:(j + 1) * 512], **st)
                        for j in range(2):
                            nc.tensor.matmul(acc[("pq", j)][:], lhsT=w_sb["q"][:, i, :],
                                             rhs=xt[:, j * 512:(j + 1) * 512], **st)
                        for j in range(2):
                            nc.tensor.matmul(acc[("pv", j)][:], lhsT=w_sb["v"][:, i, :],
                                             rhs=xt[:, j * 512:(j + 1) * 512], **st)
                for j in range(4):
                    sl = np.s_[:, j * 512:(j + 1) * 512]
                    nc.vector.tensor_scalar_add(kt_sb[sl], acc[("pk", j)][:], b_sb["k"][:])
                for j in range(2):
                    sl = np.s_[:, j * 512:(j + 1) * 512]
                    nc.vector.tensor_scalar_add(qt_sb[sl], acc[("pq", j)][:], b_sb["q"][:])
                    nc.vector.tensor_scalar_add(vt_sb[sl], acc[("pv", j)][:], b_sb["v"][:])

            # --- phase 2: attention + V second half, software-pipelined ---
            with (
                tc.tile_pool(name="attn_sb", bufs=3) as apool,
                tc.tile_pool(name="wt_sb", bufs=6) as wtpool,
                tc.tile_pool(name="mask_sb", bufs=4) as mpool,
                tc.tile_pool(name="stats", bufs=8) as stat,
                tc.tile_pool(name="l_psum", bufs=1, space="PSUM") as lpool,
                tc.tile_pool(name="o_psum", bufs=2, space="PSUM") as opool,
                tc.tile_pool(name="ppsum_b", bufs=1, space="PSUM") as ppb,
                tc.tile_pool(name="xin_b", bufs=4) as xpb,
                tc.tile_pool(name="out_sb", bufs=2) as ospool,
            ):
                accv = [ppb.tile([DH, 512], F32, tag=f"pv2{j}", name=f"pv2{j}")
                        for j in range(2)]

                def v2_chunk(ci):
                    # d-chunks of the V second-half projection; x tiles stream
                    # on the sync queue inside the attention window
                    for i in range(ci * 8, ci * 8 + 8):
                        xt = xpb.tile([128, SQ], BF16, tag="xt2")
                        nc.sync.dma_start(out=xt[:], in_=xT[i * 128:(i + 1) * 128, SQ:])
                        st = dict(start=(i == 0), stop=(i == D_CH - 1))
                        for j in range(2):
                            nc.tensor.matmul(accv[j][:], lhsT=w_sb["v"][:, i, :],
                                             rhs=xt[:, j * 512:(j + 1) * 512], **st)
                    if ci == 3:
                        for j in range(2):
                            sl = np.s_[:, SQ + j * 512:SQ + (j + 1) * 512]
                            nc.vector.tensor_scalar_add(vt_sb[sl], accv[j][:], b_sb["v"][:])
                        # V^T -> V via DMA xbar transpose (SBUF->SBUF)
                        nc.sync.dma_start_transpose(out=v_sb[:], in_=vt_sb[:])

                pv_args = {}

                # Causal skip with an SPMD-uniform graph: q-tile qt attends to
                # own-half key chunks 0..qt (chunks >qt are strictly above the
                # diagonal -> fully masked -> exp 0 -> contribute 0) plus all 8
                # other-half chunks. For h=0 cores the other half is entirely
                # masked (the per-core mask data zeroes it); for h=1 it is the
                # entirely-visible past. Only the diagonal chunk and the other
                # half ever need mask values; own chunks <qt are mask-free.
                def softmax_stage(qt):
                    qsl = np.s_[:, qt * 128:(qt + 1) * 128]
                    own = (qt + 1) * 128        # own-half extent in keys
                    ext = own + SQ              # total computed key extent
                    nch = qt + 1 + 8            # chunks of 128 computed
                    qrows = np.s_[qt * 128:(qt + 1) * 128]
                    mask_d = mpool.tile([128, SQ], BF16, tag="mask_d")
                    nc.gpsimd.dma_start(out=mask_d[:, :own], in_=mask[qrows, :own])
                    mask_o = mpool.tile([128, SQ], BF16, tag="mask_o")
                    nc.gpsimd.dma_start(out=mask_o[:], in_=mask[qrows, SQ:])

                    pl = lpool.tile([128, SQ], F32, tag="pl")
                    for lo in range(0, own, 512):
                        w = min(512, own - lo)
                        nc.tensor.matmul(pl[:, lo:lo + w], lhsT=qt_sb[qsl],
                                         rhs=kt_sb[:, lo:lo + w], start=True, stop=True)
                    plo = lpool.tile([128, SQ], F32, tag="plo")
                    for n in range(2):
                        nc.tensor.matmul(plo[:, n * 512:(n + 1) * 512], lhsT=qt_sb[qsl],
                                         rhs=kt_sb[:, SQ + n * 512:SQ + (n + 1) * 512],
                                         start=True, stop=True)

                    # compact lm: [0,own) = own-half, [own, ext) = other half
                    # one fused DVE add per half: f32 PSUM + bf16 mask -> bf16
                    # (the bf16 rounding matches the reference's logit dtype)
                    lm = apool.tile([128, S], BF16, tag="lm")
                    nc.vector.tensor_add(lm[:, :own], pl[:, :own], mask_d[:, :own])
                    nc.vector.tensor_add(lm[:, own:ext], plo[:], mask_o[:])
                    negmax = stat.tile([128, 1], F32, tag="negmax")
                    nc.vector.reduce_max(out=negmax[:], in_=lm[:, :ext],
                                         axis=mybir.AxisListType.X, negate=True)
                    w_t = apool.tile([128, S], BF16, tag="w")
                    sumexp = stat.tile([128, 1], F32, tag="sumexp")
                    nc.scalar.activation(
                        out=w_t[:, :ext], in_=lm[:, :ext],
                        func=mybir.ActivationFunctionType.Exp,
                        bias=negmax[:], scale=1.0, accum_out=sumexp[:],
                    )
                    wt_t = wtpool.tile([128, K_CH, 128], BF16, tag="wt")
                    nc.sync.dma_start_transpose(out=wt_t[:, :nch, :], in_=w_t[:, :ext])
                    pv_args[qt] = (wt_t, sumexp, nch)

                def pv_stage(qt):
                    wt_t, sumexp, nch = pv_args.pop(qt)
                    rsum = stat.tile([128, 1], F32, tag="rsum")
                    nc.vector.reciprocal(rsum[:], sumexp[:])
                    po = opool.tile([128, DH], F32, tag="po")
                    for c in range(nch):
                        vc = c if c <= qt else 8 + (c - qt - 1)
                        nc.tensor.matmul(po[:], lhsT=wt_t[:, c, :], rhs=v_sb[:, vc, :],
                                         start=(c == 0), stop=(c == nch - 1))
                    o_sb = ospool.tile([128, DH], BF16, tag="o")
                    nc.vector.tensor_scalar_mul(o_sb[:], po[:], rsum[:])
                    nc.gpsimd.dma_start(out=out[qt * 128:(qt + 1) * 128, :], in_=o_sb[:])

                DEPTH = 4
                for qt in range(QT_TILES):
                    softmax_stage(qt)
                    if qt < 4:
                        v2_chunk(qt)
                    if qt >= DEPTH:
                        pv_stage(qt - DEPTH)
                for qt in range(QT_TILES - DEPTH, QT_TILES):
                    pv_stage(qt)

    nc.finalize()
    return nc


def shard_inputs(x, attn_mask, Wq, bq, Wk, bk, Wv, bv):
    """Host-side shard prep. Returns in_maps for cores 0..7."""
    bf = ml_dtypes.bfloat16
    xb = np.asarray(x).astype(bf)                   # cast first, like the reference
    mask_f = np.asarray(attn_mask)

    def tile_w(W):
        # [DH, D] -> [128, D_CH, DH] with w[p, i, m] = W[m, i*128+p]
        WT = np.asarray(W).astype(bf).T.reshape(D_CH, 128, DH)
        return np.ascontiguousarray(WT.transpose(1, 0, 2))

    wqt, wkt, wvt = tile_w(Wq), tile_w(Wk), tile_w(Wv)
    bqc = np.asarray(bq).astype(bf).astype(np.float32).reshape(DH, 1)
    bkc = np.asarray(bk).astype(bf).astype(np.float32).reshape(DH, 1)
    bvc = np.asarray(bv).astype(bf).astype(np.float32).reshape(DH, 1)

    in_maps = []
    for c in range(N_CORES):
        b, h = divmod(c, 2)
        if h == 0:
            perm = np.arange(S)
        else:
            perm = np.concatenate([np.arange(SQ, S), np.arange(0, SQ)])
        xT = np.ascontiguousarray(xb[b][perm].T)                     # [D, S]
        msk = np.ascontiguousarray(
            mask_f[h * SQ:(h + 1) * SQ][:, perm].astype(bf))          # [SQ, S]
        in_maps.append({
            "xT": xT, "mask": msk,
            "wqT": wqt, "wkT": wkt, "wvT": wvt,
            "bq": bqc, "bk": bkc, "bv": bvc,
        })
    return in_maps


_NC_CACHE = {}


def kernel(x, attn_mask, Wq, bq, Wk, bk, Wv, bv):
    if "nc" not in _NC_CACHE:
        _NC_CACHE["nc"] = build_nc()
    nc = _NC_CACHE["nc"]
    in_maps = shard_inputs(x, attn_mask, Wq, bq, Wk, bk, Wv, bv)
    res = run_bass_kernel_spmd(nc, in_maps, list(range(N_CORES)))
    out = np.empty((B, S, DH), dtype=ml_dtypes.bfloat16)
    for c in range(N_CORES):
        b, h = divmod(c, 2)
        out[b, h * SQ:(h + 1) * SQ, :] = res.results[c]["out"]
    return out



# revision 8
# speedup vs baseline: 1.1430x; 1.1430x over previous
"""Trainium2 Bass kernel for a single attention head (B=4, S=2048, D=4096, DH=128).

Sharding: 8 cores = (batch b, parity h). Core (b, h) owns q-tiles {h, h+2, ...,
h+14} of its batch -- even/odd striping balances the causal triangle exactly
(each core computes 2(i+1) key-chunks for its i-th q-tile, i=0..7).

Host permutes keys per core to [own tiles | peer tiles] (each increasing), so
the SPMD graph is identical across cores:
  - pass P: K/V projections for the peer half (4 PSUM banks) while x streams,
  - pass M: K/V/Q for the own half (6 banks). x is read exactly once.
  - attention per q-tile i: own chunks 0..i + peer chunks 0..i. The only
    mask-dependent blocks are the diagonal own chunk (constant triu block T)
    and the last peer chunk (all -1e9 for h=0, zeros for h=1, constant C);
    everything else computed is fully visible. T/C live in small constant
    SBUF tiles, sliced right-aligned -- no bulk mask DMA.

Softmax: one fused tensor_tensor_reduce per half computes
  neg_lm = -(logits + mask) (bf16, matching the reference's bf16 logits) and
  chains a min-reduction = -(row max). ACT then does exp(-neg_lm + negmax)
  with a row-sum accumulator. Weights are DMA-transposed pair-interleaved
  ([128, slot, 256] for q-tile pairs) so PV runs 256-wide. The output is the
  unnormalized PV^T plus row sums; the host divides (elementwise epilogue).
"""

import numpy as np
import ml_dtypes

import concourse.bass as bass
import concourse.tile as tile
from concourse import bacc, mybir
from concourse.bass_utils import run_bass_kernel_spmd

B, S, D, DH = 4, 2048, 4096, 128
SQ = S // 2          # q rows per core
N_CORES = 8
D_CH = D // 128      # 32 contraction chunks
QT = 8               # q row tiles per core

BF16 = mybir.dt.bfloat16
F32 = mybir.dt.float32
FMAX = 3.0e38


def build_nc():
    nc = bacc.Bacc(None)

    xT = nc.dram_tensor("xT", [D, S], BF16, kind="ExternalInput")
    # weights pre-tiled on host: w[p, i, m] = W[m, i*128+p]
    wqT = nc.dram_tensor("wqT", [128, D_CH, DH], BF16, kind="ExternalInput")
    wkT = nc.dram_tensor("wkT", [128, D_CH, DH], BF16, kind="ExternalInput")
    wvT = nc.dram_tensor("wvT", [128, D_CH, DH], BF16, kind="ExternalInput")
    bq = nc.dram_tensor("bq", [DH, 1], F32, kind="ExternalInput")
    bk = nc.dram_tensor("bk", [DH, 1], F32, kind="ExternalInput")
    bv = nc.dram_tensor("bv", [DH, 1], F32, kind="ExternalInput")
    maskT = nc.dram_tensor("maskT", [128, 128], BF16, kind="ExternalInput")
    maskC = nc.dram_tensor("maskC", [128, 128], BF16, kind="ExternalInput")
    outT = nc.dram_tensor("outT", [DH, SQ], BF16, kind="ExternalOutput")
    sums = nc.dram_tensor("sums", [128, QT], F32, kind="ExternalOutput")

    with tile.TileContext(nc) as tc:
        with (
            tc.tile_pool(name="weights", bufs=1) as wpool,
            tc.tile_pool(name="persist", bufs=1) as persist,
        ):
            w_sb = {}
            for name in ("q", "k", "v"):
                w_sb[name] = wpool.tile([128, D_CH, DH], BF16, tag=f"w{name}",
                                        name=f"w{name}")
            # k/v first: pass P needs them from chunk 0; q before pass M
            for sl in range(4):
                for name, ext in (("k", wkT), ("v", wvT), ("q", wqT)):
                    ss = np.s_[:, sl * 8:(sl + 1) * 8, :]
                    nc.scalar.dma_start(out=w_sb[name][ss], in_=ext[ss])
            b_sb = {}
            for name, ext in (("k", bk), ("v", bv), ("q", bq)):
                t = wpool.tile([DH, 1], F32, tag=f"b{name}")
                nc.scalar.dma_start(out=t[:], in_=ext[:])
                b_sb[name] = t

            kt_sb = persist.tile([DH, S], BF16, tag="kt")    # K^T [own|peer]
            vt_sb = persist.tile([DH, S], BF16, tag="vt")    # V^T [own|peer]
            qt_sb = persist.tile([DH, SQ], BF16, tag="qt")   # Q^T (own)
            v_sb = persist.tile([128, 16, DH], BF16, tag="v")  # V chunked
            sums_sb = persist.tile([128, QT], F32, tag="sums")

            # constant mask tiles: [zeros(896) | block(128)], sliced from the
            # right so the block always lands on the last computed chunk
            zt_sb = persist.tile([128, SQ], BF16, tag="zt")
            zc_sb = persist.tile([128, SQ], BF16, tag="zc")
            nc.gpsimd.memset(zt_sb[:, :SQ - 128], 0.0)
            nc.gpsimd.memset(zc_sb[:, :SQ - 128], 0.0)
            nc.scalar.dma_start(out=zt_sb[:, SQ - 128:], in_=maskT[:])
            nc.scalar.dma_start(out=zc_sb[:, SQ - 128:], in_=maskC[:])

            # --- pass P: K/V for the peer half (x cols 1024:2048) ---
            with tc.tile_pool(name="psum_p", bufs=1, space="PSUM") as ppp:
                acc = {}
                for tag in ("pk0", "pk1", "pv0", "pv1"):
                    acc[tag] = ppp.tile([DH, 512], F32, tag=tag, name=tag)
                with tc.tile_pool(name="xin_p", bufs=6) as xpp:
                    for i in range(D_CH):
                        xt = xpp.tile([128, SQ], BF16, tag="xt")
                        nc.sync.dma_start(out=xt[:], in_=xT[i * 128:(i + 1) * 128, SQ:])
                        st = dict(start=(i == 0), stop=(i == D_CH - 1))
                        for j in range(2):
                            nc.tensor.matmul(acc[f"pk{j}"][:], lhsT=w_sb["k"][:, i, :],
                                             rhs=xt[:, j * 512:(j + 1) * 512], **st)
                        for j in range(2):
                            nc.tensor.matmul(acc[f"pv{j}"][:], lhsT=w_sb["v"][:, i, :],
                                             rhs=xt[:, j * 512:(j + 1) * 512], **st)
                for j in range(2):
                    sl = np.s_[:, SQ + j * 512:SQ + (j + 1) * 512]
                    nc.vector.tensor_scalar_add(kt_sb[sl], acc[f"pk{j}"][:], b_sb["k"][:])
                    nc.vector.tensor_scalar_add(vt_sb[sl], acc[f"pv{j}"][:], b_sb["v"][:])
                # peer-half V chunks -> slots 8..15 (overlaps pass M)
                nc.sync.dma_start_transpose(out=v_sb[:, 8:16, :], in_=vt_sb[:, SQ:])

            # --- pass M: K/V/Q for the own half (x cols 0:1024) ---
            with tc.tile_pool(name="psum_m", bufs=1, space="PSUM") as ppm:
                acc = {}
                for tag in ("mk0", "mk1", "mv0", "mv1", "mq0", "mq1"):
                    acc[tag] = ppm.tile([DH, 512], F32, tag=tag, name=tag)
                with tc.tile_pool(name="xin_m", bufs=6) as xpm:
                    for i in range(D_CH):
                        xt = xpm.tile([128, SQ], BF16, tag="xt")
                        nc.sync.dma_start(out=xt[:], in_=xT[i * 128:(i + 1) * 128, :SQ])
                        st = dict(start=(i == 0), stop=(i == D_CH - 1))
                        for j in range(2):
                            nc.tensor.matmul(acc[f"mk{j}"][:], lhsT=w_sb["k"][:, i, :],
                                             rhs=xt[:, j * 512:(j + 1) * 512], **st)
                        for j in range(2):
                            nc.tensor.matmul(acc[f"mv{j}"][:], lhsT=w_sb["v"][:, i, :],
                                             rhs=xt[:, j * 512:(j + 1) * 512], **st)
                        for j in range(2):
                            nc.tensor.matmul(acc[f"mq{j}"][:], lhsT=w_sb["q"][:, i, :],
                                             rhs=xt[:, j * 512:(j + 1) * 512], **st)
                for j in range(2):
                    sl = np.s_[:, j * 512:(j + 1) * 512]
                    nc.vector.tensor_scalar_add(kt_sb[sl], acc[f"mk{j}"][:], b_sb["k"][:])
                    nc.vector.tensor_scalar_add(qt_sb[sl], acc[f"mq{j}"][:], b_sb["q"][:])
                    nc.vector.tensor_scalar_add(vt_sb[sl], acc[f"mv{j}"][:], b_sb["v"][:])
                nc.sync.dma_start_transpose(out=v_sb[:, 0:8, :], in_=vt_sb[:, :SQ])

            # --- attention, software-pipelined over q-tiles ---
            with (
                tc.tile_pool(name="lm_sb", bufs=2) as lmpool,
                tc.tile_pool(name="w_sb2", bufs=2) as wepool,
                tc.tile_pool(name="wt_sb", bufs=2) as wtpool,
                tc.tile_pool(name="o_sb", bufs=2) as opool,
                tc.tile_pool(name="stats", bufs=8) as stat,
                tc.tile_pool(name="l_psum", bufs=1, space="PSUM") as lpool,
                tc.tile_pool(name="o_psum", bufs=2, space="PSUM") as popool,
            ):
                pl_own = lpool.tile([128, SQ], F32, tag="pl_own", name="pl_own")
                pl_peer = lpool.tile([128, SQ], F32, tag="pl_peer", name="pl_peer")
                pair_bufs = {}

                def soft(i):
                    e = (i + 1) * 128
                    qsl = np.s_[:, i * 128:(i + 1) * 128]
                    p = i // 2
                    if i % 2 == 0:
                        wt = wtpool.tile([128, 16, 256], BF16, tag="wt")
                        pair_bufs[p] = wt
                        # slots tile 2p doesn't cover (its pair-half only)
                        nc.gpsimd.memset(wt[:, i + 1:i + 2, 0:128], 0.0)
                        nc.gpsimd.memset(wt[:, 9 + i:10 + i, 0:128], 0.0)
                    else:
                        wt = pair_bufs[p]
                    half = np.s_[(i % 2) * 128:(i % 2) * 128 + 128]

                    for lo in range(0, e, 512):
                        w = min(512, e - lo)
                        nc.tensor.matmul(pl_own[:, lo:lo + w], lhsT=qt_sb[qsl],
                                         rhs=kt_sb[:, lo:lo + w], start=True, stop=True)
                    for lo in range(0, e, 512):
                        w = min(512, e - lo)
                        nc.tensor.matmul(pl_peer[:, lo:lo + w], lhsT=qt_sb[qsl],
                                         rhs=kt_sb[:, SQ + lo:SQ + lo + w],
                                         start=True, stop=True)

                    lm = lmpool.tile([128, S], BF16, tag="lm")
                    nc.vector.tensor_add(lm[:, :e], pl_own[:, :e], zt_sb[:, SQ - e:])
                    nc.vector.tensor_add(lm[:, e:2 * e], pl_peer[:, :e],
                                         zc_sb[:, SQ - e:])
                    negmax = stat.tile([128, 1], F32, tag="negmax")
                    nc.vector.reduce_max(out=negmax[:], in_=lm[:, :2 * e],
                                         axis=mybir.AxisListType.X, negate=True)

                    w_t = wepool.tile([128, S], BF16, tag="w")
                    nc.scalar.activation(
                        out=w_t[:, :2 * e], in_=lm[:, :2 * e],
                        func=mybir.ActivationFunctionType.Exp,
                        bias=negmax[:], scale=1.0, accum_out=sums_sb[:, i:i + 1])

                    nc.sync.dma_start_transpose(out=wt[:, 0:i + 1, half],
                                                in_=w_t[:, :e])
                    nc.sync.dma_start_transpose(out=wt[:, 8:9 + i, half],
                                                in_=w_t[:, e:2 * e])

                def pv(p):
                    wt = pair_bufs.pop(p)
                    hi = 2 * p + 1  # odd tile of the pair
                    slots = list(range(0, hi + 1)) + list(range(8, 9 + hi))
                    po = popool.tile([128, 256], F32, tag="poT")
                    for n, s in enumerate(slots):
                        nc.tensor.matmul(po[:], lhsT=v_sb[:, s, :], rhs=wt[:, s, :],
                                         start=(n == 0), stop=(n == len(slots) - 1))
                    o_sb = opool.tile([128, 256], BF16, tag="o")
                    nc.vector.tensor_copy(o_sb[:], po[:])
                    nc.gpsimd.dma_start(out=outT[:, p * 256:(p + 1) * 256], in_=o_sb[:])

                for i in range(QT):
                    soft(i)
                    if i in (3, 4, 5):
                        pv(i - 3)
                pv(3)
                nc.gpsimd.dma_start(out=sums[:], in_=sums_sb[:])

    nc.finalize()
    return nc


def shard_inputs(x, attn_mask, Wq, bq, Wk, bk, Wv, bv):
    """Host-side shard prep. Returns in_maps for cores 0..7."""
    bf = ml_dtypes.bfloat16
    xb = np.asarray(x).astype(bf)                   # cast first, like the reference
    mask_f = np.asarray(attn_mask)

    def tile_w(W):
        # [DH, D] -> [128, D_CH, DH] with w[p, i, m] = W[m, i*128+p]
        WT = np.asarray(W).astype(bf).T.reshape(D_CH, 128, DH)
        return np.ascontiguousarray(WT.transpose(1, 0, 2))

    wqt, wkt, wvt = tile_w(Wq), tile_w(Wk), tile_w(Wv)
    bqc = np.asarray(bq).astype(bf).astype(np.float32).reshape(DH, 1)
    bkc = np.asarray(bk).astype(bf).astype(np.float32).reshape(DH, 1)
    bvc = np.asarray(bv).astype(bf).astype(np.float32).reshape(DH, 1)

    # constant mask blocks (causal structure: all diagonal blocks equal; all
    # first-superdiagonal blocks equal; all subdiagonal blocks equal)
    mT = np.ascontiguousarray(mask_f[0:128, 0:128].astype(bf))
    mC = {0: np.ascontiguousarray(mask_f[0:128, 128:256].astype(bf)),
          1: np.ascontiguousarray(mask_f[128:256, 0:128].astype(bf))}

    in_maps = []
    for c in range(N_CORES):
        b, h = divmod(c, 2)
        own = np.concatenate([np.arange(t * 128, (t + 1) * 128)
                              for t in range(h, 16, 2)])
        peer = np.concatenate([np.arange(t * 128, (t + 1) * 128)
                               for t in range(1 - h, 16, 2)])
        perm = np.concatenate([own, peer])
        xT = np.ascontiguousarray(xb[b][perm].T)                     # [D, S]
        in_maps.append({
            "xT": xT, "maskT": mT, "maskC": mC[h],
            "wqT": wqt, "wkT": wkt, "wvT": wvt,
            "bq": bqc, "bk": bkc, "bv": bvc,
        })
    return in_maps


def unshard(core_out):
    """core_out: list of dicts with 'outT' [DH, SQ] bf16 and 'sums' [128, QT]."""
    out = np.empty((B, S, DH), dtype=ml_dtypes.bfloat16)
    for c in range(N_CORES):
        b, h = divmod(c, 2)
        oT = np.asarray(core_out[c]["outT"], dtype=np.float32)
        sm = np.asarray(core_out[c]["sums"], dtype=np.float32)
        for j in range(QT):
            t = h + 2 * j
            blk = oT[:, j * 128:(j + 1) * 128] / sm[:, j][None, :]
            out[b, t * 128:(t + 1) * 128, :] = blk.T.astype(ml_dtypes.bfloat16)
    return out


_NC_CACHE = {}


def kernel(x, attn_mask, Wq, bq, Wk, bk, Wv, bv):
    if "nc" not in _NC_CACHE:
        _NC_CACHE["nc"] = build_nc()
    nc = _NC_CACHE["nc"]
    in_maps = shard_inputs(x, attn_mask, Wq, bq, Wk, bk, Wv, bv)
    res = run_bass_kernel_spmd(nc, in_maps, list(range(N_CORES)))
    return unshard(res.results)
